# revision 12
# baseline (speedup 1.0000x reference)
"""CKConv (nn_CKConv_85950885527678) Trainium2 Bass kernel.

Strategy: data-parallel over batch (8 batches -> 8 NeuronCores). The tiny
SIREN kernel network (~134 MFLOP) is evaluated on the host and the generated
conv kernel is replicated to every core in a matmul-ready layout (as the
sharding hint suggests: "replicate the tiny SIREN params and generated
kernel").

Per core the causal conv out[o,t] = sum_{i,l} K[o,i,l] * xpad[i,t+l]
(xpad = x left-padded with T zeros, taps l in [1,2048]; l=0 never
contributes) is computed as a block-triangular matmul:
  - taps grouped into 512 blocks of 4; contraction K = 128 = (4 taps x 32 in
    channels) per matmul
  - W[dd*32+i, blk*32+o] = K[o,i,4*blk+1+dd]  (stationary operand)
  - XP[dd*32+i, c] = xpad[i, U0+c+dd]         (moving operand; a single
    shifted-replicated copy serves every tap block via column offsets)
  - 4 tap blocks run concurrently in the four 32-wide PE column groups
    (tile_position), accumulating in 4 psum partition ranges; a constant
    S = tile(I_32, 4x1) matmul reduces the groups, then bias is added.
"""

import os
import numpy as np

B, C_IN, C_OUT, T, D = 8, 32, 32, 2048, 32
L = T + 1
NBLK = 512
U0 = 1534
XPW = 2565
NT = 4
N_CORES = 8

_cache = {}


# ---------------------------------------------------------------- host prep

def _siren_kernel(pos_rel, w1, b1, w2, b2, w3, b3):
    p = pos_rel.reshape(1, L).astype(np.float32)
    h = np.sin(w1.astype(np.float32) @ p + b1[:, None].astype(np.float32))
    h = np.sin(w2.astype(np.float32) @ h + b2[:, None].astype(np.float32))
    k = w3.astype(np.float32) @ h + b3[:, None].astype(np.float32)
    return k.astype(np.float32)


def _build_w(k):
    kk = k.reshape(C_OUT, C_IN, L)[:, :, 1:]
    arr = kk.reshape(C_OUT, C_IN, NBLK, 4)
    return np.ascontiguousarray(
        arr.transpose(3, 1, 2, 0).reshape(128, NBLK * C_OUT)
    ).astype(np.float32)


def _build_xp(x):
    xpad = np.zeros((B, C_IN, 2 * T + 8), np.float32)
    xpad[:, :, T : 2 * T] = x
    XP = np.empty((B, 128, XPW), np.float32)
    for dd in range(4):
        XP[:, dd * 32 : (dd + 1) * 32, :] = xpad[:, :, U0 + dd : U0 + dd + XPW]
    return XP


# ------------------------------------------------------- tile drain patch

def _patch_tile_drain():
    """This walrus build rejects >2 sync waits on a CTRL (Drain) instruction;
    spread the TileContext exit waits over single-wait NOPs instead."""
    from concourse.tile import TileContext
    from concourse.vector_clock import ScopedClock, VectorClock

    if getattr(TileContext, "_ck_drain_patched", False):
        return

    def _drain_and_barrier(self, tick_clock, wait_clock):
        gc = tick_clock.global_clock
        n = len(gc)
        for p in range(n):
            if gc[p] <= 0:
                continue
            vec = [gc[q] if q == p else 0 for q in range(n)]
            nop = self.nc.sync.nop(nofuse=True, hint=f"split_drain_wait_p{p}")
            wait_clock.add_sem_waits(nop.ins, ScopedClock({None: VectorClock(vec)}))
        self.nc.sync.drain()
        self.nc.all_engine_barrier()
        assert self.sems is not None
        popped = self.nc._tile_sem_poison_stack.pop()
        assert popped is self._sem_poison
        self.nc.clear_and_free_semaphores(list(self.sems.allocated().values()))
        self.nc.all_engine_barrier()

    TileContext._drain_and_barrier = _drain_and_barrier
    TileContext._ck_drain_patched = True


WAIT_LIMIT = 1  # this walrus build encodes at most 2 sync waits per instruction


def _split_excess_waits(nc, limit=WAIT_LIMIT):
    """Hoist excess sem waits onto same-engine NOPs placed just before the
    instruction — in-order engine queues make this semantically identical."""
    import concourse.mybir as mybir

    n_split = 0
    for f in nc.m.functions:
        for bb in f.blocks:
            new_insts = []
            changed = False
            for inst in bb.instructions:
                si = inst.sync_info
                waits = list(si.on_wait) if si is not None and si.on_wait else []
                if len(waits) > limit:
                    extra, keep = waits[:-limit], waits[-limit:]
                    for i in range(0, len(extra), limit):
                        n_split += 1
                        new_insts.append(
                            mybir.InstNoOp(
                                name=f"I-ckwsplit-{n_split}",
                                engine=inst.engine,
                                ins=[],
                                outs=[],
                                sync_info=mybir.SyncInfo(
                                    on_wait=extra[i : i + limit], on_update=[]
                                ),
                            )
                        )
                    inst.sync_info = mybir.SyncInfo(
                        on_wait=keep, on_update=list(si.on_update) if si.on_update else []
                    )
                    changed = True
                new_insts.append(inst)
            if changed:
                bb.instructions = new_insts
    return n_split


# ------------------------------------------------------------ device kernel

def _build_nc(mm_dtype_name):
    import concourse.bass as bass
    import concourse.mybir as mybir
    from concourse.tile import TileContext

    _patch_tile_drain()
    f32 = mybir.dt.float32
    mm_dt = getattr(mybir.dt, mm_dtype_name)

    nc = bass.Bass()
    xp_d = nc.declare_dram_parameter("xp", [128, XPW], mm_dt, isOutput=False)
    w_d = nc.declare_dram_parameter("w", [128, NBLK * 32], mm_dt, isOutput=False)
    s_d = nc.declare_dram_parameter("s", [128, 32], f32, isOutput=False)
    bias_d = nc.declare_dram_parameter("bias", [32, 1], f32, isOutput=False)
    out_d = nc.declare_dram_parameter("out", [32, T], f32, isOutput=True)

    # float32r matmuls reject tile_position on this walrus -> untiled path
    tiled = mm_dtype_name != "float32r"

    with TileContext(nc) as tc:
        with (
            tc.tile_pool(name="const", bufs=1) as const,
            tc.tile_pool(name="work", bufs=3) as work,
            tc.tile_pool(name="acc_psum", bufs=2, space="PSUM") as acc_psum,
            tc.tile_pool(name="red_psum", bufs=2, space="PSUM") as red_psum,
        ):
            xp_sb = const.tile([128, XPW], mm_dt)
            nc.sync.dma_start(xp_sb[:, :], xp_d[:, :])
            s_sb = const.tile([128, 32], f32)
            nc.sync.dma_start(s_sb[:, :], s_d[:, :])
            bias_sb = const.tile([32, 1], f32)
            nc.sync.dma_start(bias_sb[:, :], bias_d[:, :])
            w_sb = const.tile([128, NBLK * 32], mm_dt)
            # chunk order matches first use: tile j=0 needs blk>=384 first
            for lo, hi in [(384, 512), (256, 384), (128, 256), (0, 128)]:
                nc.sync.dma_start(w_sb[:, lo * 32 : hi * 32], w_d[:, lo * 32 : hi * 32])

            max_tiles = int(os.environ.get("CK_MAX_TILES", str(NT)))
            max_rounds = int(os.environ.get("CK_MAX_ROUNDS", "99999"))
            for j in range(max_tiles):
                t0 = 512 * j
                blo = 128 * (3 - j)
                rounds = list(range(blo, NBLK, 4))[:max_rounds]
                last_r = len(rounds) - 1
                if tiled:
                    acc = acc_psum.tile([128, 512], f32)
                    for r, blk0 in enumerate(rounds):
                        for g in range(4):
                            blk = blk0 + g
                            off = t0 + 4 * blk + 1 - U0
                            nc.tensor.matmul(
                                acc[32 * g : 32 * (g + 1), :],
                                w_sb[:, 32 * blk : 32 * blk + 32],
                                xp_sb[:, off : off + 512],
                                start=(r == 0),
                                stop=(r == last_r),
                                tile_position=(0, 32 * g),
                            )
                    red = work.tile([128, 512], f32)
                    nc.vector.tensor_copy(red[:, :], acc[:, :])
                    ps2 = red_psum.tile([32, 512], f32)
                    nc.tensor.matmul(ps2[:, :], s_sb[:, :], red[:, :],
                                     start=True, stop=True)
                    ot = work.tile([32, 512], f32)
                    nc.vector.tensor_scalar_add(ot[:, :], ps2[:, :], bias_sb[:, :])
                else:
                    acc = acc_psum.tile([32, 512], f32)
                    blks = [b for r0 in rounds for b in range(r0, r0 + 4)]
                    nblks = len(blks)
                    for r, blk in enumerate(blks):
                        off = t0 + 4 * blk + 1 - U0
                        nc.tensor.matmul(
                            acc[:, :],
                            w_sb[:, 32 * blk : 32 * blk + 32],
                            xp_sb[:, off : off + 512],
                            start=(r == 0),
                            stop=(r == nblks - 1),
                        )
                    ot = work.tile([32, 512], f32)
                    nc.vector.tensor_scalar_add(ot[:, :], acc[:, :], bias_sb[:, :])
                nc.sync.dma_start(out_d[:, t0 : t0 + 512], ot[:, :])
    _split_excess_waits(nc)
    return nc


# ------------------------------------------------------------------- entry

def kernel(**inputs):
    from concourse.bass_utils import run_bass_kernel_spmd

    x = np.asarray(inputs["x"], dtype=np.float32)
    k = _siren_kernel(
        np.asarray(inputs["pos_rel"]), np.asarray(inputs["w1"]),
        np.asarray(inputs["b1"]), np.asarray(inputs["w2"]),
        np.asarray(inputs["b2"]), np.asarray(inputs["w3"]),
        np.asarray(inputs["b3"]),
    )
    W = _build_w(k)
    XP = _build_xp(x)
    S = np.tile(np.eye(C_OUT, dtype=np.float32), (4, 1))
    bias = np.ascontiguousarray(
        np.asarray(inputs["bias"], dtype=np.float32).reshape(32, 1)
    )

    mm_dtype = os.environ.get("CK_MM_DTYPE", "bfloat16")
    if "nc" not in _cache or _cache.get("mm_dtype") != mm_dtype:
        _cache["nc"] = _build_nc(mm_dtype)
        _cache["mm_dtype"] = mm_dtype
    nc = _cache["nc"]

    if mm_dtype == "bfloat16":
        import ml_dtypes

        W = W.astype(ml_dtypes.bfloat16)
        XP = XP.astype(ml_dtypes.bfloat16)

    n_cores = int(os.environ.get("CK_CORES", str(N_CORES)))
    in_maps = [
        {"xp": XP[b % B], "w": W, "s": S, "bias": bias} for b in range(n_cores)
    ]
    res = run_bass_kernel_spmd(nc, in_maps, core_ids=list(range(n_cores)))
    out = np.stack(
        [res.results[b % n_cores]["out"] for b in range(B)], axis=0
    )
    return out.astype(np.float32)


# revision 14
# speedup vs baseline: 3532.2560x; 3532.2560x over previous
"""CKConv (nn_CKConv_85950885527678) Trainium2 Bass kernel.

Strategy: data-parallel over batch (8 batches -> 8 NeuronCores). The tiny
SIREN kernel network (~134 MFLOP) is evaluated on the host and the generated
conv kernel is replicated to every core in a matmul-ready layout (as the
sharding hint suggests: "replicate the tiny SIREN params and generated
kernel").

Per core the causal conv out[o,t] = sum_{i,l} K[o,i,l] * xpad[i,t+l]
(xpad = x left-padded with T zeros, taps l in [1,2048]; l=0 never
contributes) is computed as a block-triangular matmul:
  - taps grouped into 512 blocks of 4; contraction K = 128 = (4 taps x 32 in
    channels) per matmul
  - W[dd*32+i, blk*32+o] = K[o,i,4*blk+1+dd]  (stationary operand)
  - XP[dd*32+i, c] = xpad[i, U0+c+dd]         (moving operand; a single
    shifted-replicated copy serves every tap block via column offsets)
  - 4 tap blocks run concurrently in the four 32-wide PE column groups
    (tile_position), accumulating in 4 psum partition ranges; a constant
    S = tile(I_32, 4x1) matmul reduces the groups, then bias is added.
"""

import os
import numpy as np

B, C_IN, C_OUT, T, D = 8, 32, 32, 2048, 32
L = T + 1
NBLK = 512
U0 = 1534
XPW = 2565
NT = 4
N_CORES = 8

_cache = {}


# ---------------------------------------------------------------- host prep

def _siren_kernel(pos_rel, w1, b1, w2, b2, w3, b3):
    p = pos_rel.reshape(1, L).astype(np.float32)
    h = np.sin(w1.astype(np.float32) @ p + b1[:, None].astype(np.float32))
    h = np.sin(w2.astype(np.float32) @ h + b2[:, None].astype(np.float32))
    k = w3.astype(np.float32) @ h + b3[:, None].astype(np.float32)
    return k.astype(np.float32)


def _build_w(k):
    kk = k.reshape(C_OUT, C_IN, L)[:, :, 1:]
    arr = kk.reshape(C_OUT, C_IN, NBLK, 4)
    return np.ascontiguousarray(
        arr.transpose(3, 1, 2, 0).reshape(128, NBLK * C_OUT)
    ).astype(np.float32)


def _build_xp(x):
    xpad = np.zeros((B, C_IN, 2 * T + 8), np.float32)
    xpad[:, :, T : 2 * T] = x
    XP = np.empty((B, 128, XPW), np.float32)
    for dd in range(4):
        XP[:, dd * 32 : (dd + 1) * 32, :] = xpad[:, :, U0 + dd : U0 + dd + XPW]
    return XP


# ------------------------------------------------------- tile drain patch

def _patch_tile_drain():
    """This walrus build rejects >2 sync waits on a CTRL (Drain) instruction;
    spread the TileContext exit waits over single-wait NOPs instead."""
    from concourse.tile import TileContext
    from concourse.vector_clock import ScopedClock, VectorClock

    if getattr(TileContext, "_ck_drain_patched", False):
        return

    def _drain_and_barrier(self, tick_clock, wait_clock):
        gc = tick_clock.global_clock
        n = len(gc)
        for p in range(n):
            if gc[p] <= 0:
                continue
            vec = [gc[q] if q == p else 0 for q in range(n)]
            nop = self.nc.sync.nop(nofuse=True, hint=f"split_drain_wait_p{p}")
            wait_clock.add_sem_waits(nop.ins, ScopedClock({None: VectorClock(vec)}))
        self.nc.sync.drain()
        self.nc.all_engine_barrier()
        assert self.sems is not None
        popped = self.nc._tile_sem_poison_stack.pop()
        assert popped is self._sem_poison
        self.nc.clear_and_free_semaphores(list(self.sems.allocated().values()))
        self.nc.all_engine_barrier()

    TileContext._drain_and_barrier = _drain_and_barrier
    TileContext._ck_drain_patched = True


WAIT_LIMIT = 1  # this walrus build encodes at most 2 sync waits per instruction


def _split_excess_waits(nc, limit=WAIT_LIMIT):
    """Hoist excess sem waits onto same-engine NOPs placed just before the
    instruction — in-order engine queues make this semantically identical."""
    import concourse.mybir as mybir

    n_split = 0
    for f in nc.m.functions:
        for bb in f.blocks:
            new_insts = []
            changed = False
            for inst in bb.instructions:
                si = inst.sync_info
                waits = list(si.on_wait) if si is not None and si.on_wait else []
                if len(waits) > limit:
                    extra, keep = waits[:-limit], waits[-limit:]
                    for i in range(0, len(extra), limit):
                        n_split += 1
                        new_insts.append(
                            mybir.InstNoOp(
                                name=f"I-ckwsplit-{n_split}",
                                engine=inst.engine,
                                ins=[],
                                outs=[],
                                sync_info=mybir.SyncInfo(
                                    on_wait=extra[i : i + limit], on_update=[]
                                ),
                            )
                        )
                    inst.sync_info = mybir.SyncInfo(
                        on_wait=keep, on_update=list(si.on_update) if si.on_update else []
                    )
                    changed = True
                new_insts.append(inst)
            if changed:
                bb.instructions = new_insts
    return n_split


# ------------------------------------------------------------ device kernel

def _build_nc(mm_dtype_name):
    import concourse.bass as bass
    import concourse.mybir as mybir
    from concourse.tile import TileContext

    _patch_tile_drain()
    f32 = mybir.dt.float32
    mm_dt = getattr(mybir.dt, mm_dtype_name)

    nc = bass.Bass()
    xp_d = nc.declare_dram_parameter("xp", [128, XPW], mm_dt, isOutput=False)
    w_d = nc.declare_dram_parameter("w", [128, NBLK * 32], mm_dt, isOutput=False)
    s_d = nc.declare_dram_parameter("s", [128, 32], f32, isOutput=False)
    bias_d = nc.declare_dram_parameter("bias", [32, 1], f32, isOutput=False)
    out_d = nc.declare_dram_parameter("out", [32, T], f32, isOutput=True)

    # float32r matmuls reject tile_position on this walrus -> untiled path
    tiled = mm_dtype_name != "float32r"

    with TileContext(nc) as tc:
        with (
            tc.tile_pool(name="const", bufs=1) as const,
            tc.tile_pool(name="work", bufs=3) as work,
            tc.tile_pool(name="acc_psum", bufs=4, space="PSUM") as acc_psum,
            tc.tile_pool(name="red_psum", bufs=2, space="PSUM") as red_psum,
        ):
            xp_sb = const.tile([128, XPW], mm_dt)
            nc.sync.dma_start(xp_sb[:, :], xp_d[:, :])
            s_sb = const.tile([128, 32], f32)
            nc.sync.dma_start(s_sb[:, :], s_d[:, :])
            bias_sb = const.tile([32, 1], f32)
            nc.sync.dma_start(bias_sb[:, :], bias_d[:, :])
            w_sb = const.tile([128, NBLK * 32], mm_dt)
            # chunk order matches first use: tile j=0 needs blk>=384 first
            for lo, hi in [(384, 512), (256, 384), (128, 256), (0, 128)]:
                nc.sync.dma_start(w_sb[:, lo * 32 : hi * 32], w_d[:, lo * 32 : hi * 32])

            max_tiles = int(os.environ.get("CK_MAX_TILES", str(NT)))
            max_rounds = int(os.environ.get("CK_MAX_ROUNDS", "99999"))
            for j in range(max_tiles):
                t0 = 512 * j
                blo = 128 * (3 - j)
                rounds = list(range(blo, NBLK, 4))[:max_rounds]
                last_r = len(rounds) - 1
                if tiled:
                    acc = acc_psum.tile([128, 512], f32)
                    for r, blk0 in enumerate(rounds):
                        for g in range(4):
                            blk = blk0 + g
                            off = t0 + 4 * blk + 1 - U0
                            nc.tensor.matmul(
                                acc[32 * g : 32 * (g + 1), :],
                                w_sb[:, 32 * blk : 32 * blk + 32],
                                xp_sb[:, off : off + 512],
                                start=(r == 0),
                                stop=(r == last_r),
                                tile_position=(0, 32 * g),
                            )
                    red = work.tile([128, 512], f32)
                    nc.vector.tensor_copy(red[:, :], acc[:, :])
                    ps2 = red_psum.tile([32, 512], f32)
                    nc.tensor.matmul(ps2[:, :], s_sb[:, :], red[:, :],
                                     start=True, stop=True)
                    ot = work.tile([32, 512], f32)
                    nc.vector.tensor_scalar_add(ot[:, :], ps2[:, :], bias_sb[:, :])
                else:
                    acc = acc_psum.tile([32, 512], f32)
                    blks = [b for r0 in rounds for b in range(r0, r0 + 4)]
                    nblks = len(blks)
                    for r, blk in enumerate(blks):
                        off = t0 + 4 * blk + 1 - U0
                        nc.tensor.matmul(
                            acc[:, :],
                            w_sb[:, 32 * blk : 32 * blk + 32],
                            xp_sb[:, off : off + 512],
                            start=(r == 0),
                            stop=(r == nblks - 1),
                        )
                    ot = work.tile([32, 512], f32)
                    nc.vector.tensor_scalar_add(ot[:, :], acc[:, :], bias_sb[:, :])
                nc.sync.dma_start(out_d[:, t0 : t0 + 512], ot[:, :])
    _split_excess_waits(nc)
    return nc


# ------------------------------------------------------------------- entry

def kernel(**inputs):
    from concourse.bass_utils import run_bass_kernel_spmd

    x = np.asarray(inputs["x"], dtype=np.float32)
    k = _siren_kernel(
        np.asarray(inputs["pos_rel"]), np.asarray(inputs["w1"]),
        np.asarray(inputs["b1"]), np.asarray(inputs["w2"]),
        np.asarray(inputs["b2"]), np.asarray(inputs["w3"]),
        np.asarray(inputs["b3"]),
    )
    W = _build_w(k)
    XP = _build_xp(x)
    S = np.tile(np.eye(C_OUT, dtype=np.float32), (4, 1))
    bias = np.ascontiguousarray(
        np.asarray(inputs["bias"], dtype=np.float32).reshape(32, 1)
    )

    mm_dtype = os.environ.get("CK_MM_DTYPE", "bfloat16")
    if "nc" not in _cache or _cache.get("mm_dtype") != mm_dtype:
        _cache["nc"] = _build_nc(mm_dtype)
        _cache["mm_dtype"] = mm_dtype
    nc = _cache["nc"]

    if mm_dtype == "bfloat16":
        import ml_dtypes

        W = W.astype(ml_dtypes.bfloat16)
        XP = XP.astype(ml_dtypes.bfloat16)

    n_cores = int(os.environ.get("CK_CORES", str(N_CORES)))
    in_maps = [
        {"xp": XP[b % B], "w": W, "s": S, "bias": bias} for b in range(n_cores)
    ]

    # The axon-tunneled device occasionally throws a transient
    # NRT_EXEC_UNIT_UNRECOVERABLE on 8-core launches; retry, then fall back
    # to two 4-core waves (same NEFF, batches split across waves).
    res = None
    for attempt in range(3):
        try:
            res = run_bass_kernel_spmd(nc, in_maps, core_ids=list(range(n_cores)))
            break
        except Exception:
            if attempt == 2:
                res = None
            else:
                continue
    if res is not None:
        out = np.stack(
            [res.results[b % n_cores]["out"] for b in range(B)], axis=0
        )
        return out.astype(np.float32)

    half = n_cores // 2 if n_cores > 1 else 1
    outs = []
    for w0 in range(0, B, half):
        wave_maps = [
            {"xp": XP[(w0 + c) % B], "w": W, "s": S, "bias": bias}
            for c in range(half)
        ]
        wres = run_bass_kernel_spmd(nc, wave_maps, core_ids=list(range(half)))
        outs.extend(wres.results[c]["out"] for c in range(half))
    out = np.stack(outs[:B], axis=0)
    return out.astype(np.float32)
